# revision 13
# baseline (speedup 1.0000x reference)
r"""DetCon (NT-Xent style) contrastive loss on 8 Trainium2 NeuronCores.

Reference computes, for v0/v1 = L2-normalized (over E) views scaled by
1/sqrt(T):   logits = [[S01, S00\diag], [S10, S11\diag]]  (2BN x 2BN-1)
             loss = mean_i( logsumexp(row_i) - label_logit_i )
with label_logit_i = S01[i,i] (== S10[i,i]).

Per-core plan (data-parallel over rows, host np.roll makes the program
core-independent):
  - load both views in natural [E, B*N] layout (2 x [128, 4096] f32 halves)
  - squares (gpsimd) -> column sumsq via ones-matmul (PE) -> per-column
    scale = exp(-0.5*ln(sumsq)) * sqrt(10) (ACT) -> partition-broadcast via
    K=1 matmul (PE) -> scale+downcast to bf16 (DVE)
  - 256 bf16 matmuls [128,512] (K=256) -> PSUM [128,2048] tiles
  - ACT exp with accum_out = fused row-sums; DVE tensor_tensor_reduce with
    identity extracts label/diag values from PSUM
  - rowsum -= exp(diag_same_view)  (exact removal of the j==i term)
  - nll = ln(rowsum) - label; partition-reduce via ones-matmul -> scalar
Host sums the 8 per-core partial sums and divides by 2*B*N.
"""

import math
from contextlib import ExitStack

import numpy as np

import concourse.bacc as bacc
import concourse.bass as bass
import concourse.tile as tile
from concourse import mybir
from concourse.bass_utils import run_bass_kernel_spmd

B, E, N = 64, 256, 64
BN = B * N            # 4096 rows per view
NCORES = 8
CHUNK = BN // NCORES  # 512 rows (of each view) per core
P = 128
KH = E // P           # 2 contraction halves
G = 2048              # column group width (PSUM tile free dim)
NG = BN // G          # 2 column groups
TEMP = 0.1
# exp(-0.5*ln(s) + BIAS) = sqrt(10)/sqrt(s)
SCALE_BIAS = -0.5 * math.log(TEMP)

F32 = mybir.dt.float32
BF16 = mybir.dt.bfloat16


def _build_nc():
    nc = bacc.Bacc()
    vin = [
        nc.dram_tensor("view0", [B, E, N], F32, kind="ExternalInput"),
        nc.dram_tensor("view1", [B, E, N], F32, kind="ExternalInput"),
    ]
    ident_in = nc.dram_tensor("ident", [P, P], F32, kind="ExternalInput")
    out_dram = nc.dram_tensor("out", [1, 1], F32, kind="ExternalOutput")

    with ExitStack() as ctx:
        tc = ctx.enter_context(tile.TileContext(nc))
        raw_p = ctx.enter_context(tc.tile_pool(name="raw", bufs=1))
        sq_p = ctx.enter_context(tc.tile_pool(name="sq", bufs=2))
        nrm_p = ctx.enter_context(tc.tile_pool(name="nrm", bufs=1))
        vec_p = ctx.enter_context(tc.tile_pool(name="vec", bufs=2))
        scl_p = ctx.enter_context(tc.tile_pool(name="scl", bufs=2))
        esc_p = ctx.enter_context(tc.tile_pool(name="esc", bufs=2))
        dsc_p = ctx.enter_context(tc.tile_pool(name="dsc", bufs=2))
        sml_p = ctx.enter_context(tc.tile_pool(name="sml", bufs=1))
        psum_p = ctx.enter_context(tc.tile_pool(name="psum", bufs=2, space="PSUM"))

        # constants
        ident = sml_p.tile([P, P], F32, tag="ident")
        nc.sync.dma_start(out=ident[:], in_=ident_in[:])
        ones_col = sml_p.tile([P, 1], F32, tag="ones_col")
        nc.vector.memset(ones_col[:], 1.0)
        ones_row = sml_p.tile([1, P], F32, tag="ones_row")
        nc.vector.memset(ones_row[:], 1.0)
        sbias = sml_p.tile([1, 1], F32, tag="sbias")
        nc.vector.memset(sbias[:], SCALE_BIAS)

        # stats / diag collectors
        stats = sml_p.tile([P, 32], F32, tag="stats")     # exp row-sums per (hm, t)
        diag01 = sml_p.tile([P, 8], F32, tag="diag01")    # label logits
        diag00 = sml_p.tile([P, 8], F32, tag="diag00")    # same-view diag logits

        # ---- load raw views in [E, B*N] layout (two 128-partition halves) ----
        raw = [[None] * KH for _ in range(2)]
        for v in range(2):
            for h in range(KH):
                t = raw_p.tile([P, BN], F32, tag=f"raw{v}{h}")
                src = vin[v][:, h * P:(h + 1) * P, :].rearrange("b e n -> e b n")
                nc.sync.dma_start(
                    out=t[:].rearrange("e (b n) -> e b n", b=B), in_=src)
                raw[v][h] = t

        # ---- normalize: per-column scale, apply + downcast to bf16 ----
        nrm = [[nrm_p.tile([P, BN], BF16, tag=f"nrm{v}{h}", name=f"nrm{v}{h}")
                for h in range(KH)] for v in range(2)]
        for v in range(2):
            for g in range(NG):
                gs = slice(g * G, (g + 1) * G)
                sq = [sq_p.tile([P, G], F32, tag="sq", name=f"sq{v}{g}{h}")
                      for h in range(KH)]
                for h in range(KH):
                    nc.gpsimd.tensor_mul(
                        sq[h][:], raw[v][h][:, gs], raw[v][h][:, gs])
                ss = psum_p.tile([P, G], F32, tag="ps")
                for j in range(G // 512):
                    js = slice(j * 512, (j + 1) * 512)
                    for h in range(KH):
                        nc.tensor.matmul(
                            ss[0:1, js], ones_col[:], sq[h][:, js],
                            start=(h == 0), stop=(h == KH - 1))
                lnb = vec_p.tile([1, G], F32, tag="lnb")
                nc.scalar.activation(
                    lnb[:], ss[0:1, :], mybir.ActivationFunctionType.Ln)
                scl = scl_p.tile([1, G], F32, tag="scl")
                nc.scalar.activation(
                    scl[:], lnb[:], mybir.ActivationFunctionType.Exp,
                    scale=-0.5, bias=sbias[:])
                pb = psum_p.tile([P, G], F32, tag="ps")
                for j in range(G // 512):
                    js = slice(j * 512, (j + 1) * 512)
                    nc.tensor.matmul(pb[:, js], ones_row[:], scl[0:1, js])
                for h in range(KH):
                    nc.vector.tensor_mul(nrm[v][h][:, gs], raw[v][h][:, gs], pb[:])

        # ---- main: logits row-blocks x column tiles, fused exp row-sums ----
        for half in range(2):           # 0: v0 rows, 1: v1 rows
            q = nrm[half]
            for m in range(4):          # 128-row blocks of this core's chunk
                hm = half * 4 + m
                ms = slice(m * P, (m + 1) * P)
                for t in range(4):      # 4 x 2048 logits columns
                    keys = nrm[1 - half] if t < 2 else nrm[half]
                    goff = (t % 2) * G
                    pt = psum_p.tile([P, G], F32, tag="ps")
                    for k in range(KH):
                        for j in range(G // 512):
                            js = slice(j * 512, (j + 1) * 512)
                            nc.tensor.matmul(
                                pt[:, js], q[k][:, ms],
                                keys[k][:, goff + j * 512: goff + (j + 1) * 512],
                                start=(k == 0), stop=(k == KH - 1))
                    if t == 0 or t == 2:
                        # t==0: label logit (cross-view diag); t==2: same-view
                        # diag (to be removed from the row-sum later)
                        dst = diag01 if t == 0 else diag00
                        dsc = dsc_p.tile([P, P], F32, tag="dsc",
                                         name=f"dsc{hm}{t}")
                        nc.vector.tensor_mul(dsc[:], ident[:], pt[:, ms])
                        nc.vector.tensor_reduce(
                            dst[:, hm:hm + 1], dsc[:],
                            axis=mybir.AxisListType.X, op=mybir.AluOpType.add)
                    esc = esc_p.tile([P, G], BF16, tag="esc")
                    nc.scalar.activation(
                        esc[:], pt[:, :], mybir.ActivationFunctionType.Exp,
                        accum_out=stats[:, hm * 4 + t: hm * 4 + t + 1])

        # ---- epilogue: nll partial sum ----
        ediag = sml_p.tile([P, 8], F32, tag="ediag")
        nc.scalar.activation(ediag[:], diag00[:], mybir.ActivationFunctionType.Exp)
        rows = sml_p.tile([P, 8], F32, tag="rows")
        nc.vector.tensor_reduce(
            rows[:], stats[:].rearrange("p (m t) -> p m t", t=4),
            axis=mybir.AxisListType.X, op=mybir.AluOpType.add)
        nc.vector.tensor_sub(rows[:], rows[:], ediag[:])
        lnr = sml_p.tile([P, 8], F32, tag="lnr")
        lnsum = sml_p.tile([P, 1], F32, tag="lnsum")
        nc.scalar.activation(
            lnr[:], rows[:], mybir.ActivationFunctionType.Ln, accum_out=lnsum[:])
        dsum = sml_p.tile([P, 1], F32, tag="dsum")
        nc.vector.tensor_reduce(
            dsum[:], diag01[:], axis=mybir.AxisListType.X, op=mybir.AluOpType.add)
        tot = sml_p.tile([P, 1], F32, tag="tot")
        nc.vector.tensor_sub(tot[:], lnsum[:], dsum[:])
        fp = psum_p.tile([P, G], F32, tag="ps")
        nc.tensor.matmul(fp[0:1, 0:1], tot[:], ones_col[:])
        res = sml_p.tile([1, 1], F32, tag="res")
        nc.vector.tensor_copy(res[:], fp[0:1, 0:1])
        nc.sync.dma_start(out=out_dram[:], in_=res[:])

    nc.compile()
    return nc


_NC_CACHE = None


def _run_spmd(view0: np.ndarray, view1: np.ndarray, **spmd_kwargs):
    global _NC_CACHE
    if _NC_CACHE is None:
        _NC_CACHE = _build_nc()
    nc = _NC_CACHE

    ident = np.eye(P, dtype=np.float32)
    in_maps = []
    for c in range(NCORES):
        in_maps.append({
            "view0": np.ascontiguousarray(np.roll(view0, -c * (B // NCORES), axis=0)),
            "view1": np.ascontiguousarray(np.roll(view1, -c * (B // NCORES), axis=0)),
            "ident": ident,
        })
    res = run_bass_kernel_spmd(nc, in_maps, core_ids=list(range(NCORES)),
                               **spmd_kwargs)
    total = sum(float(r["out"][0, 0]) for r in res.results)
    return np.float32(total / (2 * BN)), res


def kernel(view0: np.ndarray, view1: np.ndarray) -> np.ndarray:
    loss, _ = _run_spmd(view0, view1)
    return loss


# revision 17
# speedup vs baseline: 3.7940x; 3.7940x over previous
r"""DetCon (NT-Xent style) contrastive loss on 8 Trainium2 NeuronCores.

Reference computes, for v0/v1 = L2-normalized (over E) views scaled by
1/sqrt(T):   logits = [[S01, S00\diag], [S10, S11\diag]]  (2BN x 2BN-1)
             loss = mean_i( logsumexp(row_i) - label_logit_i )
with label_logit_i = S01[i,i] (== S10[i,i]).

Per-core plan (data-parallel over rows, host np.roll makes the program
core-independent):
  - load both views in natural [E, B*N] layout (2 x [128, 4096] f32 halves)
  - squares (gpsimd) -> column sumsq via ones-matmul (PE) -> per-column
    scale = exp(-0.5*ln(sumsq)) * sqrt(10) (ACT) -> partition-broadcast via
    K=1 matmul (PE) -> scale+downcast to bf16 (DVE)
  - 256 bf16 matmuls [128,512] (K=256) -> PSUM [128,2048] tiles
  - ACT exp with accum_out = fused row-sums; DVE mult+reduce with identity
    extracts label/diag values from PSUM
  - rowsum -= exp(diag_same_view)  (exact removal of the j==i term)
  - nll = ln(rowsum) - label; partition-reduce via ones-matmul -> scalar
Host sums the 8 per-core partial sums and divides by 2*B*N.
"""

import math
from contextlib import ExitStack

import numpy as np

import concourse.bacc as bacc
import concourse.bass as bass
import concourse.tile as tile
from concourse import mybir
from concourse.bass_utils import run_bass_kernel_spmd

B, E, N = 64, 256, 64
BN = B * N            # 4096 rows per view
NCORES = 8
CHUNK = BN // NCORES  # 512 rows (of each view) per core
P = 128
KH = E // P           # 2 contraction halves
G = 2048              # column group width (PSUM tile free dim)
NG = BN // G          # 2 column groups
TEMP = 0.1
# exp(-0.5*ln(s) + BIAS) = sqrt(10)/sqrt(s)
SCALE_BIAS = -0.5 * math.log(TEMP)

F32 = mybir.dt.float32
BF16 = mybir.dt.bfloat16


def _emit_pass(nc, pl, vin, out_dram, r):
    """Emit one full loss computation (rep r, for timing replication)."""
    ident, ones_col, ones_row, sbias = pl["consts"]

    # per-pass collectors
    stats = pl["sml"].tile([P, 32], F32, tag="stats", name=f"stats{r}")
    diag01 = pl["sml"].tile([P, 8], F32, tag="diag01", name=f"diag01{r}")
    diag00 = pl["sml"].tile([P, 8], F32, tag="diag00", name=f"diag00{r}")

    # ---- load raw views in [E, B*N] layout (two 128-partition halves),
    # split per column-group across both HWDGE engines ----
    raw = [[None] * KH for _ in range(2)]
    GB = B // NG  # b-range per column group
    for v in range(2):
        for h in range(KH):
            t = pl["raw"].tile([P, BN], F32, tag=f"raw{v}{h}",
                               name=f"raw{v}{h}_{r}")
            for g in range(NG):
                src = vin[v][g * GB:(g + 1) * GB, h * P:(h + 1) * P, :] \
                    .rearrange("b e n -> e b n")
                dst = t[:, g * G:(g + 1) * G].rearrange(
                    "e (b n) -> e b n", b=GB)
                eng = nc.sync if (v + h) % 2 == 0 else nc.scalar
                eng.dma_start(out=dst, in_=src)
            raw[v][h] = t

    # ---- normalize: per-column scale, apply + downcast to bf16 ----
    nrm = [[pl["nrm"].tile([P, BN], BF16, tag=f"nrm{v}{h}",
                           name=f"nrm{v}{h}_{r}")
            for h in range(KH)] for v in range(2)]
    for g in range(NG):
        for v in range(2):
            gs = slice(g * G, (g + 1) * G)
            sq = [pl["sq"].tile([P, G], F32, tag="sq", name=f"sq{v}{g}{h}_{r}")
                  for h in range(KH)]
            for h in range(KH):
                nc.gpsimd.tensor_mul(
                    sq[h][:], raw[v][h][:, gs], raw[v][h][:, gs])
            ss = pl["psum"].tile([P, G], F32, tag="ps", name=f"ss{v}{g}_{r}")
            for j in range(G // 512):
                js = slice(j * 512, (j + 1) * 512)
                for h in range(KH):
                    nc.tensor.matmul(
                        ss[0:1, js], ones_col[:], sq[h][:, js],
                        start=(h == 0), stop=(h == KH - 1))
            lnb = pl["vec"].tile([1, G], F32, tag="lnb", name=f"lnb{v}{g}_{r}")
            nc.scalar.activation(
                lnb[:], ss[0:1, :], mybir.ActivationFunctionType.Ln)
            scl = pl["scl"].tile([1, G], F32, tag="scl", name=f"scl{v}{g}_{r}")
            nc.scalar.activation(
                scl[:], lnb[:], mybir.ActivationFunctionType.Exp,
                scale=-0.5, bias=sbias[:])
            pb = pl["psum"].tile([P, G], F32, tag="ps", name=f"pb{v}{g}_{r}")
            for j in range(G // 512):
                js = slice(j * 512, (j + 1) * 512)
                nc.tensor.matmul(pb[:, js], ones_row[:], scl[0:1, js])
            for h in range(KH):
                nc.vector.tensor_mul(nrm[v][h][:, gs], raw[v][h][:, gs], pb[:])

    # ---- main: logits row-blocks x column tiles, fused exp row-sums.
    # Column-group outer so group-0 logits overlap group-1 normalize. ----
    for g in range(NG):
        goff = g * G
        for half in range(2):       # 0: v0 rows, 1: v1 rows
            q = nrm[half]
            for m in range(4):      # 128-row blocks of this core's chunk
                hm = half * 4 + m
                ms = slice(m * P, (m + 1) * P)
                for tg in range(2):  # 0: cross-view keys, 1: same-view
                    keys = nrm[1 - half] if tg == 0 else nrm[half]
                    pt = pl["psum"].tile([P, G], F32, tag="ps",
                                         name=f"pt{g}{hm}{tg}_{r}")
                    for k in range(KH):
                        for j in range(G // 512):
                            js = slice(j * 512, (j + 1) * 512)
                            nc.tensor.matmul(
                                pt[:, js], q[k][:, ms],
                                keys[k][:, goff + j * 512:
                                        goff + (j + 1) * 512],
                                start=(k == 0), stop=(k == KH - 1))
                    if g == 0:
                        # tg==0: label logit (cross-view diag); tg==1:
                        # same-view diag (removed from row-sum later)
                        dst = diag01 if tg == 0 else diag00
                        dsc = pl["dsc"].tile([P, P], F32, tag="dsc",
                                             name=f"dsc{hm}{tg}_{r}")
                        nc.vector.tensor_mul(dsc[:], ident[:], pt[:, ms])
                        nc.vector.tensor_reduce(
                            dst[:, hm:hm + 1], dsc[:],
                            axis=mybir.AxisListType.X,
                            op=mybir.AluOpType.add)
                    esc = pl["esc"].tile([P, G], BF16, tag="esc",
                                         name=f"esc{g}{hm}{tg}_{r}")
                    sidx = hm * 4 + tg * 2 + g
                    nc.scalar.activation(
                        esc[:], pt[:, :], mybir.ActivationFunctionType.Exp,
                        accum_out=stats[:, sidx:sidx + 1])

    # ---- epilogue: nll partial sum ----
    ediag = pl["sml"].tile([P, 8], F32, tag="ediag", name=f"ediag{r}")
    nc.scalar.activation(ediag[:], diag00[:], mybir.ActivationFunctionType.Exp)
    rows = pl["sml"].tile([P, 8], F32, tag="rows", name=f"rows{r}")
    nc.vector.tensor_reduce(
        rows[:], stats[:].rearrange("p (m t) -> p m t", t=4),
        axis=mybir.AxisListType.X, op=mybir.AluOpType.add)
    nc.vector.tensor_sub(rows[:], rows[:], ediag[:])
    lnr = pl["sml"].tile([P, 8], F32, tag="lnr", name=f"lnr{r}")
    lnsum = pl["sml"].tile([P, 1], F32, tag="lnsum", name=f"lnsum{r}")
    nc.scalar.activation(
        lnr[:], rows[:], mybir.ActivationFunctionType.Ln, accum_out=lnsum[:])
    dsum = pl["sml"].tile([P, 1], F32, tag="dsum", name=f"dsum{r}")
    nc.vector.tensor_reduce(
        dsum[:], diag01[:], axis=mybir.AxisListType.X, op=mybir.AluOpType.add)
    tot = pl["sml"].tile([P, 1], F32, tag="tot", name=f"tot{r}")
    nc.vector.tensor_sub(tot[:], lnsum[:], dsum[:])
    fp = pl["psum"].tile([P, G], F32, tag="ps", name=f"fp{r}")
    nc.tensor.matmul(fp[0:1, 0:1], tot[:], ones_col[:])
    res = pl["sml"].tile([1, 1], F32, tag="res", name=f"res{r}")
    nc.vector.tensor_copy(res[:], fp[0:1, 0:1])
    nc.sync.dma_start(out=out_dram[:], in_=res[:])


def _build_nc(reps: int = 1):
    nc = bacc.Bacc()
    vin = [
        nc.dram_tensor("view0", [B, E, N], F32, kind="ExternalInput"),
        nc.dram_tensor("view1", [B, E, N], F32, kind="ExternalInput"),
    ]
    ident_in = nc.dram_tensor("ident", [P, P], F32, kind="ExternalInput")
    out_dram = nc.dram_tensor("out", [1, 1], F32, kind="ExternalOutput")

    with ExitStack() as ctx:
        tc = ctx.enter_context(tile.TileContext(nc))
        pl = {
            name: ctx.enter_context(tc.tile_pool(name=name, bufs=bufs))
            for name, bufs in (("raw", 1), ("sq", 2), ("nrm", 1), ("vec", 2),
                               ("scl", 2), ("esc", 2), ("dsc", 2), ("sml", 1))
        }
        pl["psum"] = ctx.enter_context(
            tc.tile_pool(name="psum", bufs=2, space="PSUM"))

        ident = pl["sml"].tile([P, P], F32, tag="ident", name="ident")
        nc.sync.dma_start(out=ident[:], in_=ident_in[:])
        ones_col = pl["sml"].tile([P, 1], F32, tag="ones_col", name="ones_col")
        nc.vector.memset(ones_col[:], 1.0)
        ones_row = pl["sml"].tile([1, P], F32, tag="ones_row", name="ones_row")
        nc.vector.memset(ones_row[:], 1.0)
        sbias = pl["sml"].tile([1, 1], F32, tag="sbias", name="sbias")
        nc.vector.memset(sbias[:], SCALE_BIAS)
        pl["consts"] = (ident, ones_col, ones_row, sbias)

        for r in range(reps):
            _emit_pass(nc, pl, vin, out_dram, r)

    nc.compile()
    return nc


_NC_CACHE = None


def _run_spmd(view0: np.ndarray, view1: np.ndarray, nc=None, **spmd_kwargs):
    global _NC_CACHE
    if nc is None:
        if _NC_CACHE is None:
            _NC_CACHE = _build_nc()
        nc = _NC_CACHE

    ident = np.eye(P, dtype=np.float32)
    in_maps = []
    for c in range(NCORES):
        in_maps.append({
            "view0": np.ascontiguousarray(np.roll(view0, -c * (B // NCORES), axis=0)),
            "view1": np.ascontiguousarray(np.roll(view1, -c * (B // NCORES), axis=0)),
            "ident": ident,
        })
    res = run_bass_kernel_spmd(nc, in_maps, core_ids=list(range(NCORES)),
                               **spmd_kwargs)
    total = sum(float(r["out"][0, 0]) for r in res.results)
    return np.float32(total / (2 * BN)), res


def kernel(view0: np.ndarray, view1: np.ndarray) -> np.ndarray:
    loss, _ = _run_spmd(view0, view1)
    return loss
